# revision 1
# baseline (speedup 1.0000x reference)
"""AttentionBlock (GroupNorm + 1x1-conv QKV self-attention + out-proj + residual)
as a distributed Bass kernel on 8 TRN2 NeuronCores.

Sharding: fully data-parallel, zero collectives.
  core = 2*b + half   (b in 0..3 batch element, half in 0..1 query-row half)
Each core:
  - GroupNorm stats for its batch element (duplicated within a pair;
    cheaper than exchanging k/v via collectives). The GN affine h = x*A + B
    is FOLDED into the projections: q/k/v weights are scaled by 64*A on
    device (cast to fp8e4; the 64 keeps the ~0.02-magnitude weights inside
    fp8 normal range and is divided back out of the psum copies) and B is
    folded into the projection biases with tiny matmuls, so the projections
    consume raw fp8-cast x directly — no GN apply pass at all.
  - k, vT projections over all 4096 tokens; q projection over its 2048 rows.
    All projection and attention matmuls are fp8e4 with DoubleRow perf mode
    (2x PE throughput); accumulation is always fp32 PSUM.
  - attention in transposed layout: s_T[m, n] = sum_c k[c,m] q[c,n]
    -> exp on ScalarE (fp8 out) -> PV accumulation with vT slices as the
    stationary operand, two m-chunks per DoubleRow matmul. The softmax
    denominator rides the same loop as a DoubleRow ones-matmul into a
    dedicated PSUM bank; normalization and the v-bias are deferred past
    the PV matmul (osb = PV*1/Z + bv, exact since softmax weights sum
    to 1), then out-proj (bf16), +bias +residual in fp32.
  - No on-chip transposes anywhere (vT comes from using x as the
    stationary matmul operand; weights pre-transposed on host).
"""

import os
import sys

import numpy as np

for p in ("/opt/trn_rl_repo", "/opt/pypackages"):
    if p not in sys.path:
        sys.path.append(p)

import ml_dtypes

import concourse.bass as bass
import concourse.bacc as bacc
import concourse.tile as tile
from concourse import mybir
from concourse.bass import ts
from concourse.bass_utils import run_bass_kernel_spmd

F32 = mybir.dt.float32
BF16 = mybir.dt.bfloat16
FP8 = mybir.dt.float8e4
AF = mybir.ActivationFunctionType
OP = mybir.AluOpType

# attention (S and PV) matmul dtype: fp8e4 + DoubleRow, or bf16 fallback
ATT_FP8 = True

C = 512
N = 4096
NHALF = 2048
P = 128
CCH = C // P          # 4 channel chunks
NB = N // 512         # 8 column blocks of 512
NBH = NHALF // 512    # 4
MC = N // P           # 32 key chunks of 128
EPS = 1e-5
SCALE = C ** -0.5

LAST_EXEC_TIME_NS = None

_CACHED_NC = None
_last_in_maps = None


def build_nc():
    nc = bacc.Bacc(None, target_bir_lowering=False)

    x_full = nc.declare_dram_parameter("x_full", [CCH, P, N], F32, isOutput=False)
    x_my = nc.declare_dram_parameter("x_my", [CCH, P, NHALF], F32, isOutput=False)
    wq_p = nc.declare_dram_parameter("wqT", [P, CCH, C], BF16, isOutput=False)
    wk_p = nc.declare_dram_parameter("wkT", [P, CCH, C], BF16, isOutput=False)
    wv_p = nc.declare_dram_parameter("wvT", [P, CCH, C], BF16, isOutput=False)
    wo_p = nc.declare_dram_parameter("woT", [P, CCH, C], BF16, isOutput=False)
    bq_p = nc.declare_dram_parameter("bq", [P, CCH], F32, isOutput=False)
    bk_p = nc.declare_dram_parameter("bk", [P, CCH], F32, isOutput=False)
    bo_p = nc.declare_dram_parameter("bo", [P, CCH], F32, isOutput=False)
    bv_p = nc.declare_dram_parameter("bvc", [P, CCH], F32, isOutput=False)
    gnw_p = nc.declare_dram_parameter("gnw", [P, CCH], F32, isOutput=False)
    gnb_p = nc.declare_dram_parameter("gnb", [P, CCH], F32, isOutput=False)
    ones8_p = nc.declare_dram_parameter("ones8", [P, 2, P], FP8, isOutput=False)
    ind_p = nc.declare_dram_parameter("ind", [P, 8], F32, isOutput=False)
    ind2_p = nc.declare_dram_parameter("ind2", [8, P], F32, isOutput=False)
    out_p = nc.declare_dram_parameter("out", [CCH, P, NHALF], F32, isOutput=True)

    with tile.TileContext(nc) as tc:
        ADT = FP8 if ATT_FP8 else BF16
        with tc.tile_pool(name="singles", bufs=1) as singles:
            k_t = singles.tile([P, CCH, N], ADT)
            q_t = singles.tile([P, CCH, NHALF], ADT)
            vT_t = singles.tile([P, MC, C], ADT)
            xb_t = singles.tile([P, CCH, NHALF], F32)
            A_t = singles.tile([P, CCH], F32)
            B_t = singles.tile([P, CCH], F32)
            B16_t = singles.tile([P, CCH], BF16)
            w_q = singles.tile([P, CCH, C], BF16)
            w_k = singles.tile([P, CCH, C], BF16)
            w_v = singles.tile([P, CCH, C], BF16)
            w_o = singles.tile([P, CCH, C], BF16)
            bq2_t = singles.tile([P, CCH], F32)
            bk2_t = singles.tile([P, CCH], F32)
            bv2c_t = singles.tile([P, CCH], F32)
            bq_t = singles.tile([P, CCH], F32)
            bk_t = singles.tile([P, CCH], F32)
            bo_t = singles.tile([P, CCH], F32)
            bv_t = singles.tile([P, CCH], F32)
            gnw_t = singles.tile([P, CCH], F32)
            gnb_t = singles.tile([P, CCH], F32)
            ones8_t = singles.tile([P, 2, P], FP8)
            ind_t = singles.tile([P, 8], F32)
            ind2_t = singles.tile([8, P], F32)
            eps_t = singles.tile([P, 1], F32)
            zero_t = singles.tile([P, 1], F32)
            nc.vector.memset(eps_t, EPS)
            nc.vector.memset(zero_t, 0.0)

            nc.sync.dma_start(out=w_q, in_=wq_p[:])
            nc.sync.dma_start(out=w_k, in_=wk_p[:])
            nc.sync.dma_start(out=w_v, in_=wv_p[:])
            nc.sync.dma_start(out=w_o, in_=wo_p[:])
            nc.sync.dma_start(out=bq_t, in_=bq_p[:])
            nc.sync.dma_start(out=bk_t, in_=bk_p[:])
            nc.sync.dma_start(out=bo_t, in_=bo_p[:])
            nc.sync.dma_start(out=bv_t, in_=bv_p[:])
            nc.sync.dma_start(out=gnw_t, in_=gnw_p[:])
            nc.sync.dma_start(out=gnb_t, in_=gnb_p[:])
            nc.sync.dma_start(out=ones8_t, in_=ones8_p[:])
            nc.sync.dma_start(out=ind_t, in_=ind_p[:])
            nc.sync.dma_start(out=ind2_t, in_=ind2_p[:])

            # fp8 copies of x + fp8 GN-scaled weights, alive through phase B
            with tc.tile_pool(name="xcast", bufs=1) as xcast:
                xb16 = xcast.tile([P, CCH, N], FP8)
                xq16 = xcast.tile([P, CCH, NHALF], FP8)
                w8q = xcast.tile([P, CCH, C], FP8)
                w8k = xcast.tile([P, CCH, C], FP8)
                w8v = xcast.tile([P, CCH, C], FP8)
                A64_t = xcast.tile([P, CCH], F32)

                # ---------- Phase A: GroupNorm statistics + weight folding --
                with (
                    tc.tile_pool(name="astat", bufs=4) as statp,
                    tc.tile_pool(name="aload", bufs=4) as aload,
                    tc.tile_pool(name="apsum", bufs=2, space="PSUM") as app,
                ):
                    mvall = statp.tile([P, CCH, 2], F32, tag="mvall")
                    for ci in range(CCH):
                        st6 = statp.tile([P, NB, 6], F32, tag="st6")
                        for nq in range(2):  # two 2048-wide loads per chunk
                            xt = aload.tile([P, 2048], F32, tag="xt")
                            nc.sync.dma_start(
                                out=xt, in_=x_full[ci, :, ts(nq, 2048)]
                            )
                            for sb in range(4):
                                nc.vector.bn_stats(
                                    out=st6[:, nq * 4 + sb, :],
                                    in_=xt[:, ts(sb, 512)],
                                )
                            # cast on GpSimd (idle in phase A; 1-input ops
                            # run at line rate there)
                            nc.gpsimd.tensor_copy(
                                out=xb16[:, ci, ts(nq, 2048)], in_=xt
                            )
                        nc.vector.bn_aggr(out=mvall[:, ci, :], in_=st6)

                    # Batched GN stat chain for all 4 chunks at once
                    # (wide ops; one indicator-matmul pair for all chunks)
                    rsall = statp.tile([P, CCH, 2], F32, tag="rsall")
                    nc.vector.tensor_mul(
                        out=rsall[:, :, 1:2], in0=mvall[:, :, 0:1], in1=mvall[:, :, 0:1]
                    )
                    nc.vector.tensor_add(
                        out=rsall[:, :, 1:2], in0=rsall[:, :, 1:2], in1=mvall[:, :, 1:2]
                    )
                    nc.vector.tensor_copy(out=rsall[:, :, 0:1], in_=mvall[:, :, 0:1])
                    gps = app.tile([8, CCH, 2], F32, tag="g", bufs=1)
                    nc.tensor.matmul(gps, lhsT=ind_t, rhs=rsall, start=True, stop=True)
                    gsb = statp.tile([8, CCH, 2], F32, tag="gsb")
                    nc.vector.tensor_copy(out=gsb, in_=gps)
                    rps = app.tile([P, CCH, 2], F32, tag="r", bufs=1)
                    nc.tensor.matmul(rps, lhsT=ind2_t, rhs=gsb, start=True, stop=True)
                    gmall = statp.tile([P, CCH], F32, tag="gmall")
                    gvall = statp.tile([P, CCH], F32, tag="gvall")
                    nc.vector.tensor_copy(out=gmall, in_=rps[:, :, 0:1])
                    nc.vector.tensor_mul(out=gvall, in0=gmall, in1=gmall)
                    nc.vector.tensor_sub(out=gvall, in0=rps[:, :, 1:2], in1=gvall)
                    # rstd = 1/sqrt(var + eps)
                    nc.scalar.activation(out=gvall, in_=gvall, func=AF.Sqrt, bias=eps_t)
                    nc.vector.reciprocal(out=gvall, in_=gvall)
                    nc.vector.tensor_mul(out=A_t, in0=gvall, in1=gnw_t)
                    nc.vector.tensor_mul(out=gmall, in0=gmall, in1=A_t)
                    nc.vector.tensor_sub(out=B_t, in0=gnb_t, in1=gmall)
                    nc.vector.tensor_scalar_mul(out=A64_t, in0=A_t, scalar1=64.0)
                    nc.vector.tensor_copy(out=B16_t, in_=B_t)
                    for wt, w8 in ((w_q, w8q), (w_k, w8k), (w_v, w8v)):
                        for ci in range(CCH):
                            nc.gpsimd.tensor_scalar_mul(
                                out=w8[:, ci, :],
                                in0=wt[:, ci, :],
                                scalar1=A64_t[:, ci : ci + 1],
                            )

                    # Fold B into projection biases:
                    #   bq2[o] = bq[o] + sum_c wqT[c,o] * B[c]   (same for bk2)
                    #   bv2c[e] = bv[e] + sum_c wvT[c,e] * B[c]
                    for (wt, b_in, b_out) in ((w_q, bq_t, bq2_t), (w_k, bk_t, bk2_t)):
                        for oj in range(CCH):
                            bc = app.tile([P, 1], F32, tag="bc", bufs=2)
                            for ci in range(CCH):
                                nc.tensor.matmul(
                                    bc,
                                    lhsT=wt[:, ci, ts(oj, P)],
                                    rhs=B16_t[:, ci : ci + 1],
                                    start=(ci == 0),
                                    stop=(ci == CCH - 1),
                                )
                            nc.vector.tensor_add(
                                out=b_out[:, oj : oj + 1],
                                in0=bc,
                                in1=b_in[:, oj : oj + 1],
                            )
                    for e4 in range(CCH):
                        bc = app.tile([P, 1], F32, tag="bc", bufs=2)
                        for ci in range(CCH):
                            nc.tensor.matmul(
                                bc,
                                lhsT=w_v[:, ci, ts(e4, P)],
                                rhs=B16_t[:, ci : ci + 1],
                                start=(ci == 0),
                                stop=(ci == CCH - 1),
                            )
                        nc.vector.tensor_add(
                            out=bv2c_t[:, e4 : e4 + 1],
                            in0=bc,
                            in1=bv_t[:, e4 : e4 + 1],
                        )

                # ---------- Phase B: projections (consume raw fp8 x) -------
                DR = mybir.MatmulPerfMode.DoubleRow
                INV64 = 1.0 / 64.0
                with (
                    tc.tile_pool(name="bload", bufs=2) as bload,
                    tc.tile_pool(name="bpsum", bufs=4, space="PSUM") as bpp,
                ):
                    # x_my: fp8 cast for q-projection + f32 (x+bo) residual
                    for ci in range(CCH):
                        xt = bload.tile([P, 2048], F32, tag="xt")
                        nc.sync.dma_start(out=xt, in_=x_my[ci, :, :])
                        nc.gpsimd.tensor_copy(out=xq16[:, ci, :], in_=xt)
                        nc.gpsimd.tensor_scalar_add(
                            out=xb_t[:, ci, :],
                            in0=xt,
                            scalar1=bo_t[:, ci : ci + 1],
                        )
                    for nb in range(NB):
                        for oj in range(CCH):
                            kp = bpp.tile([P, 512], F32, tag="pj")
                            for c2 in range(2):
                                nc.tensor.matmul(
                                    kp,
                                    lhsT=w8k[:, 2 * c2 : 2 * c2 + 2, ts(oj, P)],
                                    rhs=xb16[:, 2 * c2 : 2 * c2 + 2, ts(nb, 512)],
                                    start=(c2 == 0),
                                    stop=(c2 == 1),
                                    perf_mode=DR,
                                )
                            if oj % 2 == 0:
                                nc.scalar.activation(
                                    out=k_t[:, oj, ts(nb, 512)],
                                    in_=kp,
                                    func=AF.Identity,
                                    bias=bk2_t[:, oj : oj + 1],
                                    scale=INV64,
                                )
                            else:
                                nc.vector.tensor_scalar(
                                    out=k_t[:, oj, ts(nb, 512)],
                                    in0=kp,
                                    scalar1=INV64,
                                    scalar2=bk2_t[:, oj : oj + 1],
                                    op0=OP.mult,
                                    op1=OP.add,
                                )
                        for mj in range(4):
                            vp = bpp.tile([P, 512], F32, tag="pj")
                            for c2 in range(2):
                                nc.tensor.matmul(
                                    vp,
                                    lhsT=xb16[
                                        :, 2 * c2 : 2 * c2 + 2, ts(nb * 4 + mj, P)
                                    ],
                                    rhs=w8v[:, 2 * c2 : 2 * c2 + 2, :],
                                    start=(c2 == 0),
                                    stop=(c2 == 1),
                                    perf_mode=DR,
                                )
                            if mj % 2 == 0:
                                nc.scalar.activation(
                                    out=vT_t[:, nb * 4 + mj, :],
                                    in_=vp,
                                    func=AF.Identity,
                                    bias=zero_t,
                                    scale=INV64,
                                )
                            else:
                                nc.vector.tensor_scalar_mul(
                                    out=vT_t[:, nb * 4 + mj, :],
                                    in0=vp,
                                    scalar1=INV64,
                                )
                    for nb in range(NBH):
                        for oj in range(CCH):
                            qp = bpp.tile([P, 512], F32, tag="pj")
                            for c2 in range(2):
                                nc.tensor.matmul(
                                    qp,
                                    lhsT=w8q[:, 2 * c2 : 2 * c2 + 2, ts(oj, P)],
                                    rhs=xq16[:, 2 * c2 : 2 * c2 + 2, ts(nb, 512)],
                                    start=(c2 == 0),
                                    stop=(c2 == 1),
                                    perf_mode=DR,
                                )
                            nc.scalar.activation(
                                out=q_t[:, oj, ts(nb, 512)],
                                in_=qp,
                                func=AF.Identity,
                                bias=bq2_t[:, oj : oj + 1],
                                scale=INV64,
                            )

            # ---------- Phase C: attention + out-proj + residual ------------
            _phase_c_fp8(
                nc, tc, k_t, q_t, vT_t, w_o, xb_t, ones8_t, zero_t, bv2c_t,
                out_p,
            )

    nc.compile()
    return nc


def _phase_c_fp8(nc, tc, k_t, q_t, vT_t, w_o, xb_t, ones8_t, zero_t, bv2c_t, out_p):
    """Attention with fp8e4 DoubleRow matmuls, PV/Z processed in m-chunk
    pairs. Z is accumulated on the TensorEngine via a DoubleRow ones-matmul
    sharing the PV accumulation loop."""
    DR = mybir.MatmulPerfMode.DoubleRow
    with (
        tc.tile_pool(name="sps", bufs=3, space="PSUM") as sps,
        tc.tile_pool(name="ozp", bufs=4, space="PSUM") as ozp,
        tc.tile_pool(name="zps", bufs=1, space="PSUM") as zpsp,
        tc.tile_pool(name="att", bufs=4) as attp,
        tc.tile_pool(name="fin", bufs=3) as finp,
    ):
        total = NBH * MC
        sp_tiles = {}
        next_s = [0]

        def emit_s(t):
            ns_, mc_ = divmod(t, MC)
            sp = sps.tile([P, 512], F32, tag="s", name=f"s{t}")
            for c2 in range(2):
                nc.tensor.matmul(
                    sp,
                    lhsT=k_t[:, 2 * c2 : 2 * c2 + 2, ts(mc_, P)],
                    rhs=q_t[:, 2 * c2 : 2 * c2 + 2, ts(ns_, 512)],
                    start=(c2 == 0),
                    stop=(c2 == 1),
                    perf_mode=DR,
                )
            sp_tiles[t] = sp

        def ensure_s(upto):
            while next_s[0] < min(upto, total):
                emit_s(next_s[0])
                next_s[0] += 1

        ops = None
        zps = None
        e8 = None
        for t in range(total):
            ns, mc = divmod(t, MC)
            a, j = divmod(mc, 2)
            if mc == 0:
                ops = [
                    ozp.tile([P, 512], F32, tag="oz", name=f"o{ns}_{e4}")
                    for e4 in range(CCH)
                ]
                zps = zpsp.tile([P, 512], F32, tag="z", name=f"z{ns}")
            ensure_s(t + 3)
            if j == 0:
                e8 = attp.tile([P, 2, 512], FP8, tag="e", name=f"e{t}")
            nc.scalar.activation(
                out=e8[:, j, :],
                in_=sp_tiles.pop(t),
                func=AF.Exp,
                bias=zero_t,
                scale=SCALE,
            )
            if j == 1:
                for e4 in range(CCH):
                    nc.tensor.matmul(
                        ops[e4],
                        lhsT=vT_t[:, 2 * a : 2 * a + 2, ts(e4, P)],
                        rhs=e8,
                        start=(a == 0),
                        stop=(a == MC // 2 - 1),
                        perf_mode=DR,
                    )
                nc.tensor.matmul(
                    zps,
                    lhsT=ones8_t,
                    rhs=e8,
                    start=(a == 0),
                    stop=(a == MC // 2 - 1),
                    perf_mode=DR,
                )
            if mc == MC - 1:
                ensure_s(t + 3)
                rz = attp.tile([P, 512], F32, tag="rz", name=f"rz{ns}")
                nc.vector.reciprocal(out=rz, in_=zps)
                osb = attp.tile([P, CCH, 512], BF16, tag="osb", name=f"ob{ns}")
                for e4 in range(CCH):
                    nc.vector.tensor_mul(out=osb[:, e4, :], in0=ops[e4], in1=rz)
                    # v-projection bias, deferred past the PV matmul:
                    # osb = (PV/Z) + bv2c  (exact since sum_m p = 1)
                    nc.vector.tensor_scalar_add(
                        out=osb[:, e4, :],
                        in0=osb[:, e4, :],
                        scalar1=bv2c_t[:, e4 : e4 + 1],
                    )
                for oj in range(CCH):
                    pp = sps.tile([P, 512], F32, tag="s", name=f"pp{ns}_{oj}")
                    for e4 in range(CCH):
                        nc.tensor.matmul(
                            pp,
                            lhsT=w_o[:, e4, ts(oj, P)],
                            rhs=osb[:, e4, :],
                            start=(e4 == 0),
                            stop=(e4 == CCH - 1),
                        )
                    res = finp.tile([P, 512], F32, tag="res", name=f"r{ns}_{oj}")
                    nc.vector.tensor_add(
                        out=res, in0=pp, in1=xb_t[:, oj, ts(ns, 512)]
                    )
                    nc.sync.dma_start(out=out_p[oj, :, ts(ns, 512)], in_=res)


def _prep_consts(inputs):
    bf = ml_dtypes.bfloat16

    def wt(w):
        # w: [o, c] -> lhsT layout [c, o] chunked by c: [P, CCH, C]
        return np.ascontiguousarray(
            w.T.reshape(CCH, P, C).transpose(1, 0, 2)
        ).astype(bf)

    def colvec(b):
        return np.ascontiguousarray(b.reshape(CCH, P).T).astype(np.float32)

    ind = np.zeros((P, 8), np.float32)
    ind[np.arange(P), np.arange(P) // 16] = 1.0 / 16.0
    ind2 = np.zeros((8, P), np.float32)
    ind2[np.arange(P) // 16, np.arange(P)] = 1.0

    return {
        "wqT": wt(np.asarray(inputs["wq"], np.float32)),
        "wkT": wt(np.asarray(inputs["wk"], np.float32)),
        "wvT": wt(np.asarray(inputs["wv"], np.float32)),
        "woT": wt(np.asarray(inputs["wo"], np.float32)),
        "bq": colvec(np.asarray(inputs["bq"], np.float32)),
        "bk": colvec(np.asarray(inputs["bk"], np.float32)),
        "bo": colvec(np.asarray(inputs["bo"], np.float32)),
        "bvc": colvec(np.asarray(inputs["bv"], np.float32)),
        "gnw": colvec(np.asarray(inputs["gn_w"], np.float32)),
        "gnb": colvec(np.asarray(inputs["gn_b"], np.float32)),
        "ones8": np.ones((P, 2, P), ml_dtypes.float8_e4m3),
        "ind": ind,
        "ind2": ind2,
    }


def kernel(**inputs):
    global LAST_EXEC_TIME_NS, _CACHED_NC, _last_in_maps
    x = np.asarray(inputs["x"], np.float32)  # [4, 512, 64, 64]
    B = x.shape[0]
    assert x.shape == (4, C, 64, 64)

    if _CACHED_NC is None:
        _CACHED_NC = build_nc()
    nc = _CACHED_NC

    consts = _prep_consts(inputs)
    xf = np.ascontiguousarray(x.reshape(B, CCH, P, N))

    in_maps = []
    for core in range(8):
        b, half = core // 2, core % 2
        m = dict(consts)
        m["x_full"] = xf[b]
        m["x_my"] = np.ascontiguousarray(
            xf[b][:, :, half * NHALF : (half + 1) * NHALF]
        )
        in_maps.append(m)

    _last_in_maps = in_maps
    res = run_bass_kernel_spmd(nc, in_maps, core_ids=list(range(8)))
    LAST_EXEC_TIME_NS = res.exec_time_ns

    out = np.empty((B, C, N), np.float32)
    for core in range(8):
        b, half = core // 2, core % 2
        out[b, :, half * NHALF : (half + 1) * NHALF] = (
            res.results[core]["out"].reshape(C, NHALF)
        )
    return out.reshape(B, C, 64, 64)



# revision 2
# speedup vs baseline: 1.1867x; 1.1867x over previous
"""AttentionBlock (GroupNorm + 1x1-conv QKV self-attention + out-proj + residual)
as a distributed Bass kernel on 8 TRN2 NeuronCores.

Sharding: fully data-parallel, zero collectives.
  core = 2*b + half   (b in 0..3 batch element, half in 0..1 query-row half)

All inputs are packed host-side into a SINGLE per-core DRAM blob ([128, W]
f32).  The per-iteration dispatch cost of the PJRT/SPMD execute path scales
with the number of kernel parameters (~0.15 ms per tensor per call), which
dominates this kernel's actual on-device time, so 14 parameters -> 1.
Within the blob, each channel-chunk of x is stored [my-half | other-half]
so the q-projection can consume the same fp8 cast as k/v (token order
within a core is permuted; attention reduces over keys, so this is exact).

Each core:
  - GroupNorm stats for its batch element (duplicated within a pair;
    cheaper than exchanging k/v via collectives). The GN affine h = x*A + B
    is FOLDED into the projections: q/k/v weights are scaled by 64*A on
    device (cast to fp8e4; the 64 keeps the ~0.02-magnitude weights inside
    fp8 normal range and is divided back out of the psum copies) and B is
    folded into the projection biases with tiny matmuls, so the projections
    consume raw fp8-cast x directly — no GN apply pass at all.
  - k, vT projections over all 4096 tokens; q projection over its 2048 rows.
    All projection and attention matmuls are fp8e4 with DoubleRow perf mode
    (2x PE throughput); accumulation is always fp32 PSUM.
  - attention in transposed layout: s_T[m, n] = sum_c k[c,m] q[c,n]
    -> exp on ScalarE (fp8 out) -> PV accumulation with vT slices as the
    stationary operand, two m-chunks per DoubleRow matmul. The softmax
    denominator rides the same loop as a DoubleRow ones-matmul into a
    dedicated PSUM bank; normalization and the v-bias are deferred past
    the PV matmul (osb = PV*1/Z + bv, exact since softmax weights sum
    to 1), then out-proj (bf16), +bias +residual in fp32.
  - No on-chip transposes anywhere (vT comes from using x as the
    stationary matmul operand; weights pre-transposed on host).
"""

import os
import sys

import numpy as np

for p in ("/opt/trn_rl_repo", "/opt/pypackages"):
    if p not in sys.path:
        sys.path.append(p)

import concourse.bass as bass
import concourse.bacc as bacc
import concourse.tile as tile
from concourse import mybir
from concourse.bass import ts
from concourse.bass_utils import run_bass_kernel_spmd

F32 = mybir.dt.float32
BF16 = mybir.dt.bfloat16
FP8 = mybir.dt.float8e4
AF = mybir.ActivationFunctionType
OP = mybir.AluOpType

C = 512
N = 4096
NHALF = 2048
P = 128
CCH = C // P          # 4 channel chunks
NB = N // 512         # 8 column blocks of 512
NBH = NHALF // 512    # 4
MC = N // P           # 32 key chunks of 128
EPS = 1e-5
SCALE = C ** -0.5

# ---- blob layout (f32 columns per partition) -------------------------------
XOFF = 0                      # 4 chunks x [my 2048 | other 2048]
WQOFF = XOFF + CCH * N        # 16384, each weight: [128, CCH*C] = 2048 cols
WKOFF = WQOFF + CCH * C
WVOFF = WKOFF + CCH * C
WOOFF = WVOFF + CCH * C
BOFF = WOOFF + CCH * C        # bq bk bo bvc gnw gnb, 4 cols each
INDOFF = BOFF + 6 * CCH       # ind: 8 cols
IND2OFF = INDOFF + 8          # ind2: 128 cols (rows 0..8 used)
W = IND2OFF + P               # 24736

LAST_EXEC_TIME_NS = None

_CACHED_NC = None
_last_in_maps = None


def build_nc():
    nc = bacc.Bacc(None, target_bir_lowering=False)

    blob = nc.declare_dram_parameter("blob", [P, W], F32, isOutput=False)
    out_p = nc.declare_dram_parameter("out", [CCH, P, NHALF], F32, isOutput=True)

    with tile.TileContext(nc) as tc:
        ADT = FP8
        with tc.tile_pool(name="singles", bufs=1) as singles:
            k_t = singles.tile([P, CCH, N], ADT)
            q_t = singles.tile([P, CCH, NHALF], ADT)
            vT_t = singles.tile([P, MC, C], ADT)
            xb_t = singles.tile([P, CCH, NHALF], F32)
            A_t = singles.tile([P, CCH], F32)
            B_t = singles.tile([P, CCH], F32)
            wf_q = singles.tile([P, CCH * C], F32)
            wf_k = singles.tile([P, CCH * C], F32)
            wf_v = singles.tile([P, CCH * C], F32)
            wf_o = singles.tile([P, CCH * C], F32)
            w_o16 = singles.tile([P, CCH * C], BF16)
            bq2_t = singles.tile([P, CCH], F32)
            bk2_t = singles.tile([P, CCH], F32)
            bv2c_t = singles.tile([P, CCH], F32)
            bq_t = singles.tile([P, CCH], F32)
            bk_t = singles.tile([P, CCH], F32)
            bo_t = singles.tile([P, CCH], F32)
            bv_t = singles.tile([P, CCH], F32)
            gnw_t = singles.tile([P, CCH], F32)
            gnb_t = singles.tile([P, CCH], F32)
            ones8_t = singles.tile([P, 2, P], FP8)
            onesf_t = singles.tile([P, 2, P], F32)
            ind_t = singles.tile([P, 8], F32)
            ind2_t = singles.tile([8, P], F32)
            eps_t = singles.tile([P, 1], F32)
            zero_t = singles.tile([P, 1], F32)
            nc.vector.memset(eps_t, EPS)
            nc.vector.memset(zero_t, 0.0)
            nc.vector.memset(onesf_t, 1.0)
            nc.gpsimd.tensor_copy(out=ones8_t, in_=onesf_t)

            nc.sync.dma_start(out=wf_q, in_=blob[:, WQOFF : WQOFF + CCH * C])
            nc.sync.dma_start(out=wf_k, in_=blob[:, WKOFF : WKOFF + CCH * C])
            nc.sync.dma_start(out=wf_v, in_=blob[:, WVOFF : WVOFF + CCH * C])
            nc.sync.dma_start(out=wf_o, in_=blob[:, WOOFF : WOOFF + CCH * C])
            nc.sync.dma_start(out=bq_t, in_=blob[:, BOFF : BOFF + CCH])
            nc.sync.dma_start(out=bk_t, in_=blob[:, BOFF + CCH : BOFF + 2 * CCH])
            nc.sync.dma_start(
                out=bo_t, in_=blob[:, BOFF + 2 * CCH : BOFF + 3 * CCH]
            )
            nc.sync.dma_start(
                out=bv_t, in_=blob[:, BOFF + 3 * CCH : BOFF + 4 * CCH]
            )
            nc.sync.dma_start(
                out=gnw_t, in_=blob[:, BOFF + 4 * CCH : BOFF + 5 * CCH]
            )
            nc.sync.dma_start(
                out=gnb_t, in_=blob[:, BOFF + 5 * CCH : BOFF + 6 * CCH]
            )
            nc.sync.dma_start(out=ind_t, in_=blob[:, INDOFF : INDOFF + 8])
            nc.sync.dma_start(out=ind2_t, in_=blob[0:8, IND2OFF : IND2OFF + P])
            nc.vector.tensor_copy(out=w_o16, in_=wf_o)

            # fp8 copies of x + fp8 GN-scaled weights, alive through phase B
            with tc.tile_pool(name="xcast", bufs=1) as xcast:
                xb16 = xcast.tile([P, CCH, N], FP8)
                w8q = xcast.tile([P, CCH, C], FP8)
                w8k = xcast.tile([P, CCH, C], FP8)
                w8v = xcast.tile([P, CCH, C], FP8)
                A64_t = xcast.tile([P, CCH], F32)

                # ---------- Phase A: GroupNorm statistics + weight folding --
                with (
                    tc.tile_pool(name="astat", bufs=4) as statp,
                    tc.tile_pool(name="aload", bufs=4) as aload,
                    tc.tile_pool(name="apsum", bufs=2, space="PSUM") as app,
                ):
                    mvall = statp.tile([P, CCH, 2], F32, tag="mvall")
                    for ci in range(CCH):
                        st6 = statp.tile([P, NB, 6], F32, tag="st6")
                        for nq in range(2):  # nq=0: my half, nq=1: other half
                            xt = aload.tile([P, 2048], F32, tag="xt")
                            nc.sync.dma_start(
                                out=xt,
                                in_=blob[
                                    :,
                                    XOFF
                                    + ci * N
                                    + nq * 2048 : XOFF
                                    + ci * N
                                    + (nq + 1) * 2048,
                                ],
                            )
                            for sb in range(4):
                                nc.vector.bn_stats(
                                    out=st6[:, nq * 4 + sb, :],
                                    in_=xt[:, ts(sb, 512)],
                                )
                            # cast on GpSimd (idle in phase A; 1-input ops
                            # run at line rate there)
                            nc.gpsimd.tensor_copy(
                                out=xb16[:, ci, ts(nq, 2048)], in_=xt
                            )
                            if nq == 0:
                                # residual base for my half: x + bo
                                nc.gpsimd.tensor_scalar_add(
                                    out=xb_t[:, ci, :],
                                    in0=xt,
                                    scalar1=bo_t[:, ci : ci + 1],
                                )
                        nc.vector.bn_aggr(out=mvall[:, ci, :], in_=st6)

                    # Batched GN stat chain for all 4 chunks at once
                    # (wide ops; one indicator-matmul pair for all chunks)
                    rsall = statp.tile([P, CCH, 2], F32, tag="rsall")
                    nc.vector.tensor_mul(
                        out=rsall[:, :, 1:2], in0=mvall[:, :, 0:1], in1=mvall[:, :, 0:1]
                    )
                    nc.vector.tensor_add(
                        out=rsall[:, :, 1:2], in0=rsall[:, :, 1:2], in1=mvall[:, :, 1:2]
                    )
                    nc.vector.tensor_copy(out=rsall[:, :, 0:1], in_=mvall[:, :, 0:1])
                    gps = app.tile([8, CCH, 2], F32, tag="g", bufs=1)
                    nc.tensor.matmul(gps, lhsT=ind_t, rhs=rsall, start=True, stop=True)
                    gsb = statp.tile([8, CCH, 2], F32, tag="gsb")
                    nc.vector.tensor_copy(out=gsb, in_=gps)
                    rps = app.tile([P, CCH, 2], F32, tag="r", bufs=1)
                    nc.tensor.matmul(rps, lhsT=ind2_t, rhs=gsb, start=True, stop=True)
                    gmall = statp.tile([P, CCH], F32, tag="gmall")
                    gvall = statp.tile([P, CCH], F32, tag="gvall")
                    nc.vector.tensor_copy(out=gmall, in_=rps[:, :, 0:1])
                    nc.vector.tensor_mul(out=gvall, in0=gmall, in1=gmall)
                    nc.vector.tensor_sub(out=gvall, in0=rps[:, :, 1:2], in1=gvall)
                    # rstd = 1/sqrt(var + eps)
                    nc.scalar.activation(out=gvall, in_=gvall, func=AF.Sqrt, bias=eps_t)
                    nc.vector.reciprocal(out=gvall, in_=gvall)
                    nc.vector.tensor_mul(out=A_t, in0=gvall, in1=gnw_t)
                    nc.vector.tensor_mul(out=gmall, in0=gmall, in1=A_t)
                    nc.vector.tensor_sub(out=B_t, in0=gnb_t, in1=gmall)
                    nc.vector.tensor_scalar_mul(out=A64_t, in0=A_t, scalar1=64.0)
                    for wt, w8 in ((wf_q, w8q), (wf_k, w8k), (wf_v, w8v)):
                        for ci in range(CCH):
                            nc.gpsimd.tensor_scalar_mul(
                                out=w8[:, ci, :],
                                in0=wt[:, ts(ci, C)],
                                scalar1=A64_t[:, ci : ci + 1],
                            )

                    # Fold B into projection biases:
                    #   bq2[o] = bq[o] + sum_c wqT[c,o] * B[c]   (same for bk2)
                    #   bv2c[e] = bv[e] + sum_c wvT[c,e] * B[c]
                    for (wt, b_in, b_out) in ((wf_q, bq_t, bq2_t), (wf_k, bk_t, bk2_t)):
                        for oj in range(CCH):
                            bc = app.tile([P, 1], F32, tag="bc", bufs=2)
                            for ci in range(CCH):
                                nc.tensor.matmul(
                                    bc,
                                    lhsT=wt[:, ts(ci * CCH + oj, P)],
                                    rhs=B_t[:, ci : ci + 1],
                                    start=(ci == 0),
                                    stop=(ci == CCH - 1),
                                )
                            nc.vector.tensor_add(
                                out=b_out[:, oj : oj + 1],
                                in0=bc,
                                in1=b_in[:, oj : oj + 1],
                            )
                    for e4 in range(CCH):
                        bc = app.tile([P, 1], F32, tag="bc", bufs=2)
                        for ci in range(CCH):
                            nc.tensor.matmul(
                                bc,
                                lhsT=wf_v[:, ts(ci * CCH + e4, P)],
                                rhs=B_t[:, ci : ci + 1],
                                start=(ci == 0),
                                stop=(ci == CCH - 1),
                            )
                        nc.vector.tensor_add(
                            out=bv2c_t[:, e4 : e4 + 1],
                            in0=bc,
                            in1=bv_t[:, e4 : e4 + 1],
                        )

                # ---------- Phase B: projections (consume raw fp8 x) -------
                DR = mybir.MatmulPerfMode.DoubleRow
                INV64 = 1.0 / 64.0
                with tc.tile_pool(name="bpsum", bufs=4, space="PSUM") as bpp:
                    for nb in range(NB):
                        for oj in range(CCH):
                            kp = bpp.tile([P, 512], F32, tag="pj")
                            for c2 in range(2):
                                nc.tensor.matmul(
                                    kp,
                                    lhsT=w8k[:, 2 * c2 : 2 * c2 + 2, ts(oj, P)],
                                    rhs=xb16[:, 2 * c2 : 2 * c2 + 2, ts(nb, 512)],
                                    start=(c2 == 0),
                                    stop=(c2 == 1),
                                    perf_mode=DR,
                                )
                            if oj % 2 == 0:
                                nc.scalar.activation(
                                    out=k_t[:, oj, ts(nb, 512)],
                                    in_=kp,
                                    func=AF.Identity,
                                    bias=bk2_t[:, oj : oj + 1],
                                    scale=INV64,
                                )
                            else:
                                nc.vector.tensor_scalar(
                                    out=k_t[:, oj, ts(nb, 512)],
                                    in0=kp,
                                    scalar1=INV64,
                                    scalar2=bk2_t[:, oj : oj + 1],
                                    op0=OP.mult,
                                    op1=OP.add,
                                )
                        for mj in range(4):
                            vp = bpp.tile([P, 512], F32, tag="pj")
                            for c2 in range(2):
                                nc.tensor.matmul(
                                    vp,
                                    lhsT=xb16[
                                        :, 2 * c2 : 2 * c2 + 2, ts(nb * 4 + mj, P)
                                    ],
                                    rhs=w8v[:, 2 * c2 : 2 * c2 + 2, :],
                                    start=(c2 == 0),
                                    stop=(c2 == 1),
                                    perf_mode=DR,
                                )
                            if mj % 2 == 0:
                                nc.scalar.activation(
                                    out=vT_t[:, nb * 4 + mj, :],
                                    in_=vp,
                                    func=AF.Identity,
                                    bias=zero_t,
                                    scale=INV64,
                                )
                            else:
                                nc.vector.tensor_scalar_mul(
                                    out=vT_t[:, nb * 4 + mj, :],
                                    in0=vp,
                                    scalar1=INV64,
                                )
                    for nb in range(NBH):
                        for oj in range(CCH):
                            qp = bpp.tile([P, 512], F32, tag="pj")
                            for c2 in range(2):
                                nc.tensor.matmul(
                                    qp,
                                    lhsT=w8q[:, 2 * c2 : 2 * c2 + 2, ts(oj, P)],
                                    rhs=xb16[:, 2 * c2 : 2 * c2 + 2, ts(nb, 512)],
                                    start=(c2 == 0),
                                    stop=(c2 == 1),
                                    perf_mode=DR,
                                )
                            nc.scalar.activation(
                                out=q_t[:, oj, ts(nb, 512)],
                                in_=qp,
                                func=AF.Identity,
                                bias=bq2_t[:, oj : oj + 1],
                                scale=INV64,
                            )

            # ---------- Phase C: attention + out-proj + residual ------------
            _phase_c_fp8(
                nc, tc, k_t, q_t, vT_t, w_o16, xb_t, ones8_t, zero_t, bv2c_t,
                out_p,
            )

    nc.compile()
    return nc


def _phase_c_fp8(nc, tc, k_t, q_t, vT_t, w_o16, xb_t, ones8_t, zero_t, bv2c_t, out_p):
    """Attention with fp8e4 DoubleRow matmuls, PV/Z processed in m-chunk
    pairs. Z is accumulated on the TensorEngine via a DoubleRow ones-matmul
    sharing the PV accumulation loop."""
    DR = mybir.MatmulPerfMode.DoubleRow
    with (
        tc.tile_pool(name="sps", bufs=3, space="PSUM") as sps,
        tc.tile_pool(name="ozp", bufs=4, space="PSUM") as ozp,
        tc.tile_pool(name="zps", bufs=1, space="PSUM") as zpsp,
        tc.tile_pool(name="att", bufs=4) as attp,
        tc.tile_pool(name="fin", bufs=3) as finp,
    ):
        total = NBH * MC
        sp_tiles = {}
        next_s = [0]

        def emit_s(t):
            ns_, mc_ = divmod(t, MC)
            sp = sps.tile([P, 512], F32, tag="s", name=f"s{t}")
            for c2 in range(2):
                nc.tensor.matmul(
                    sp,
                    lhsT=k_t[:, 2 * c2 : 2 * c2 + 2, ts(mc_, P)],
                    rhs=q_t[:, 2 * c2 : 2 * c2 + 2, ts(ns_, 512)],
                    start=(c2 == 0),
                    stop=(c2 == 1),
                    perf_mode=DR,
                )
            sp_tiles[t] = sp

        def ensure_s(upto):
            while next_s[0] < min(upto, total):
                emit_s(next_s[0])
                next_s[0] += 1

        ops = None
        zps = None
        e8 = None
        for t in range(total):
            ns, mc = divmod(t, MC)
            a, j = divmod(mc, 2)
            if mc == 0:
                ops = [
                    ozp.tile([P, 512], F32, tag="oz", name=f"o{ns}_{e4}")
                    for e4 in range(CCH)
                ]
                zps = zpsp.tile([P, 512], F32, tag="z", name=f"z{ns}")
            ensure_s(t + 3)
            if j == 0:
                e8 = attp.tile([P, 2, 512], FP8, tag="e", name=f"e{t}")
            nc.scalar.activation(
                out=e8[:, j, :],
                in_=sp_tiles.pop(t),
                func=AF.Exp,
                bias=zero_t,
                scale=SCALE,
            )
            if j == 1:
                for e4 in range(CCH):
                    nc.tensor.matmul(
                        ops[e4],
                        lhsT=vT_t[:, 2 * a : 2 * a + 2, ts(e4, P)],
                        rhs=e8,
                        start=(a == 0),
                        stop=(a == MC // 2 - 1),
                        perf_mode=DR,
                    )
                nc.tensor.matmul(
                    zps,
                    lhsT=ones8_t,
                    rhs=e8,
                    start=(a == 0),
                    stop=(a == MC // 2 - 1),
                    perf_mode=DR,
                )
            if mc == MC - 1:
                ensure_s(t + 3)
                rz = attp.tile([P, 512], F32, tag="rz", name=f"rz{ns}")
                nc.vector.reciprocal(out=rz, in_=zps)
                osb = attp.tile([P, CCH, 512], BF16, tag="osb", name=f"ob{ns}")
                for e4 in range(CCH):
                    nc.vector.tensor_mul(out=osb[:, e4, :], in0=ops[e4], in1=rz)
                    # v-projection bias, deferred past the PV matmul:
                    # osb = (PV/Z) + bv2c  (exact since sum_m p = 1)
                    nc.vector.tensor_scalar_add(
                        out=osb[:, e4, :],
                        in0=osb[:, e4, :],
                        scalar1=bv2c_t[:, e4 : e4 + 1],
                    )
                for oj in range(CCH):
                    pp = sps.tile([P, 512], F32, tag="s", name=f"pp{ns}_{oj}")
                    for e4 in range(CCH):
                        nc.tensor.matmul(
                            pp,
                            lhsT=w_o16[:, ts(e4 * CCH + oj, P)],
                            rhs=osb[:, e4, :],
                            start=(e4 == 0),
                            stop=(e4 == CCH - 1),
                        )
                    res = finp.tile([P, 512], F32, tag="res", name=f"r{ns}_{oj}")
                    nc.vector.tensor_add(
                        out=res, in0=pp, in1=xb_t[:, oj, ts(ns, 512)]
                    )
                    nc.sync.dma_start(out=out_p[oj, :, ts(ns, 512)], in_=res)


def _pack_blobs(inputs):
    """Build the 8 per-core [P, W] f32 blobs."""
    x = np.asarray(inputs["x"], np.float32)  # [4, 512, 64, 64]
    B = x.shape[0]
    xf = np.ascontiguousarray(x.reshape(B, CCH, P, N))

    def wt(w):
        # w: [o, c] -> lhsT layout [c, o] chunked by c: [P, CCH*C]
        return np.ascontiguousarray(
            np.asarray(w, np.float32).T.reshape(CCH, P, C).transpose(1, 0, 2)
        ).reshape(P, CCH * C)

    def colvec(b):
        return np.ascontiguousarray(
            np.asarray(b, np.float32).reshape(CCH, P).T
        )

    const = np.zeros((P, W - WQOFF), np.float32)
    o = 0
    for nm in ("wq", "wk", "wv", "wo"):
        const[:, o : o + CCH * C] = wt(inputs[nm])
        o += CCH * C
    for nm in ("bq", "bk", "bo", "bv", "gn_w", "gn_b"):
        const[:, o : o + CCH] = colvec(inputs[nm])
        o += CCH
    ind = np.zeros((P, 8), np.float32)
    ind[np.arange(P), np.arange(P) // 16] = 1.0 / 16.0
    const[:, o : o + 8] = ind
    o += 8
    ind2 = np.zeros((8, P), np.float32)
    ind2[np.arange(P) // 16, np.arange(P)] = 1.0
    const[0:8, o : o + P] = ind2

    blobs = []
    for core in range(8):
        b, half = core // 2, core % 2
        blob = np.empty((P, W), np.float32)
        for ci in range(CCH):
            blob[:, ci * N : ci * N + NHALF] = xf[
                b, ci, :, half * NHALF : (half + 1) * NHALF
            ]
            blob[:, ci * N + NHALF : (ci + 1) * N] = xf[
                b, ci, :, (1 - half) * NHALF : (2 - half) * NHALF
            ]
        blob[:, WQOFF:] = const
        blobs.append(blob)
    return blobs


def kernel(**inputs):
    global LAST_EXEC_TIME_NS, _CACHED_NC, _last_in_maps
    x = np.asarray(inputs["x"], np.float32)  # [4, 512, 64, 64]
    B = x.shape[0]
    assert x.shape == (4, C, 64, 64)

    if _CACHED_NC is None:
        _CACHED_NC = build_nc()
    nc = _CACHED_NC

    in_maps = [{"blob": blob} for blob in _pack_blobs(inputs)]

    _last_in_maps = in_maps
    res = run_bass_kernel_spmd(nc, in_maps, core_ids=list(range(8)))
    LAST_EXEC_TIME_NS = res.exec_time_ns

    out = np.empty((B, C, N), np.float32)
    for core in range(8):
        b, half = core // 2, core % 2
        out[b, :, half * NHALF : (half + 1) * NHALF] = (
            res.results[core]["out"].reshape(C, NHALF)
        )
    return out.reshape(B, C, 64, 64)


# revision 3
# speedup vs baseline: 1.6492x; 1.3897x over previous
"""AttentionBlock (GroupNorm + 1x1-conv QKV self-attention + out-proj + residual)
as a distributed Bass kernel on 8 TRN2 NeuronCores.

Sharding: fully data-parallel, zero collectives.
  core = 2*b + half   (b in 0..3 batch element, half in 0..1 query-row half)

All inputs are packed host-side into a SINGLE per-core DRAM blob ([128, W]
f32).  The per-iteration dispatch cost of the PJRT/SPMD execute path scales
with the number of kernel parameters (~0.15 ms per tensor per call), which
dominates this kernel's actual on-device time, so 14 parameters -> 1.
Within the blob, each channel-chunk of x is stored [my-half | other-half]
so the q-projection can consume the same fp8 cast as k/v (token order
within a core is permuted; attention reduces over keys, so this is exact).

Each core:
  - GroupNorm stats for its batch element (duplicated within a pair;
    cheaper than exchanging k/v via collectives). The GN affine h = x*A + B
    is FOLDED into the projections: q/k/v weights are scaled by 64*A on
    device (cast to fp8e4; the 64 keeps the ~0.02-magnitude weights inside
    fp8 normal range and is divided back out of the psum copies) and B is
    folded into the projection biases with tiny matmuls, so the projections
    consume raw fp8-cast x directly — no GN apply pass at all.
  - k, vT projections over all 4096 tokens; q projection over its 2048 rows.
    All projection and attention matmuls are fp8e4 with DoubleRow perf mode
    (2x PE throughput); accumulation is always fp32 PSUM.
  - attention in transposed layout: s_T[m, n] = sum_c k[c,m] q[c,n]
    -> exp on ScalarE (fp8 out) -> PV accumulation with vT slices as the
    stationary operand, two m-chunks per DoubleRow matmul. The softmax
    denominator rides the same loop as a DoubleRow ones-matmul into a
    dedicated PSUM bank; normalization and the v-bias are deferred past
    the PV matmul (osb = PV*1/Z + bv, exact since softmax weights sum
    to 1), then out-proj (bf16), +bias +residual in fp32.
  - No on-chip transposes anywhere (vT comes from using x as the
    stationary matmul operand; weights pre-transposed on host).
"""

import os
import sys

import numpy as np

for p in ("/opt/trn_rl_repo", "/opt/pypackages"):
    if p not in sys.path:
        sys.path.append(p)

import concourse.bass as bass
import concourse.bacc as bacc
import concourse.tile as tile
from concourse import mybir
from concourse.bass import ts
from concourse.bass_utils import run_bass_kernel_spmd

F32 = mybir.dt.float32
BF16 = mybir.dt.bfloat16
FP8 = mybir.dt.float8e4
AF = mybir.ActivationFunctionType
OP = mybir.AluOpType

C = 512
N = 4096
NHALF = 2048
P = 128
CCH = C // P          # 4 channel chunks
NB = N // 512         # 8 column blocks of 512
NBH = NHALF // 512    # 4
MC = N // P           # 32 key chunks of 128
EPS = 1e-5
SCALE = C ** -0.5

# ---- blob layout (f32 columns per partition) -------------------------------
XOFF = 0                      # 4 chunks x [my 2048 | other 2048]
WQOFF = XOFF + CCH * N        # 16384, each weight: [128, CCH*C] = 2048 cols
WKOFF = WQOFF + CCH * C
WVOFF = WKOFF + CCH * C
WOOFF = WVOFF + CCH * C
BOFF = WOOFF + CCH * C        # bq bk bo bvc gnw gnb, 4 cols each
INDOFF = BOFF + 6 * CCH       # ind: 8 cols
IND2OFF = INDOFF + 8          # ind2: 128 cols (rows 0..8 used)
W = IND2OFF + P               # 24736

LAST_EXEC_TIME_NS = None

_CACHED_NC = None
_last_in_maps = None


def build_nc():
    nc = bacc.Bacc(None, target_bir_lowering=False, enable_partition_id=False)

    blob = nc.declare_dram_parameter("blob", [P, W], F32, isOutput=False)
    out_p = nc.declare_dram_parameter("out", [CCH, P, NHALF], F32, isOutput=True)

    with tile.TileContext(nc) as tc:
        ADT = FP8
        with tc.tile_pool(name="singles", bufs=1) as singles:
            k_t = singles.tile([P, CCH, N], ADT)
            q_t = singles.tile([P, CCH, NHALF], ADT)
            vT_t = singles.tile([P, MC, C], ADT)
            xb_t = singles.tile([P, CCH, NHALF], F32)
            A_t = singles.tile([P, CCH], F32)
            B_t = singles.tile([P, CCH], F32)
            wf_q = singles.tile([P, CCH * C], F32)
            wf_k = singles.tile([P, CCH * C], F32)
            wf_v = singles.tile([P, CCH * C], F32)
            wf_o = singles.tile([P, CCH * C], F32)
            w_o16 = singles.tile([P, CCH * C], BF16)
            bq2_t = singles.tile([P, CCH], F32)
            bk2_t = singles.tile([P, CCH], F32)
            bv2c_t = singles.tile([P, CCH], F32)
            bq_t = singles.tile([P, CCH], F32)
            bk_t = singles.tile([P, CCH], F32)
            bo_t = singles.tile([P, CCH], F32)
            bv_t = singles.tile([P, CCH], F32)
            gnw_t = singles.tile([P, CCH], F32)
            gnb_t = singles.tile([P, CCH], F32)
            ones8_t = singles.tile([P, 2, P], FP8)
            onesf_t = singles.tile([P, 2, P], F32)
            ind_t = singles.tile([P, 8], F32)
            ind2_t = singles.tile([8, P], F32)
            eps_t = singles.tile([P, 1], F32)
            zero_t = singles.tile([P, 1], F32)
            nc.vector.memset(eps_t, EPS)
            nc.vector.memset(zero_t, 0.0)
            nc.vector.memset(onesf_t, 1.0)
            nc.gpsimd.tensor_copy(out=ones8_t, in_=onesf_t)

            nc.sync.dma_start(out=wf_q, in_=blob[:, WQOFF : WQOFF + CCH * C])
            nc.sync.dma_start(out=wf_k, in_=blob[:, WKOFF : WKOFF + CCH * C])
            nc.sync.dma_start(out=wf_v, in_=blob[:, WVOFF : WVOFF + CCH * C])
            nc.sync.dma_start(out=wf_o, in_=blob[:, WOOFF : WOOFF + CCH * C])
            nc.sync.dma_start(out=bq_t, in_=blob[:, BOFF : BOFF + CCH])
            nc.sync.dma_start(out=bk_t, in_=blob[:, BOFF + CCH : BOFF + 2 * CCH])
            nc.sync.dma_start(
                out=bo_t, in_=blob[:, BOFF + 2 * CCH : BOFF + 3 * CCH]
            )
            nc.sync.dma_start(
                out=bv_t, in_=blob[:, BOFF + 3 * CCH : BOFF + 4 * CCH]
            )
            nc.sync.dma_start(
                out=gnw_t, in_=blob[:, BOFF + 4 * CCH : BOFF + 5 * CCH]
            )
            nc.sync.dma_start(
                out=gnb_t, in_=blob[:, BOFF + 5 * CCH : BOFF + 6 * CCH]
            )
            nc.sync.dma_start(out=ind_t, in_=blob[:, INDOFF : INDOFF + 8])
            nc.sync.dma_start(out=ind2_t, in_=blob[0:8, IND2OFF : IND2OFF + P])
            nc.vector.tensor_copy(out=w_o16, in_=wf_o)

            # fp8 copies of x + fp8 GN-scaled weights, alive through phase B
            with tc.tile_pool(name="xcast", bufs=1) as xcast:
                xb16 = xcast.tile([P, CCH, N], FP8)
                w8q = xcast.tile([P, CCH, C], FP8)
                w8k = xcast.tile([P, CCH, C], FP8)
                w8v = xcast.tile([P, CCH, C], FP8)
                A64_t = xcast.tile([P, CCH], F32)

                # ---------- Phase A: GroupNorm statistics + weight folding --
                with (
                    tc.tile_pool(name="astat", bufs=4) as statp,
                    tc.tile_pool(name="aload", bufs=4) as aload,
                    tc.tile_pool(name="apsum", bufs=2, space="PSUM") as app,
                ):
                    mvall = statp.tile([P, CCH, 2], F32, tag="mvall")
                    for ci in range(CCH):
                        st6 = statp.tile([P, NB, 6], F32, tag="st6")
                        for nq in range(2):  # nq=0: my half, nq=1: other half
                            xt = aload.tile([P, 2048], F32, tag="xt")
                            nc.sync.dma_start(
                                out=xt,
                                in_=blob[
                                    :,
                                    XOFF
                                    + ci * N
                                    + nq * 2048 : XOFF
                                    + ci * N
                                    + (nq + 1) * 2048,
                                ],
                            )
                            for sb in range(4):
                                nc.vector.bn_stats(
                                    out=st6[:, nq * 4 + sb, :],
                                    in_=xt[:, ts(sb, 512)],
                                )
                            # cast on GpSimd (idle in phase A; 1-input ops
                            # run at line rate there)
                            nc.gpsimd.tensor_copy(
                                out=xb16[:, ci, ts(nq, 2048)], in_=xt
                            )
                            if nq == 0:
                                # residual base for my half: x + bo
                                nc.gpsimd.tensor_scalar_add(
                                    out=xb_t[:, ci, :],
                                    in0=xt,
                                    scalar1=bo_t[:, ci : ci + 1],
                                )
                        nc.vector.bn_aggr(out=mvall[:, ci, :], in_=st6)

                    # Batched GN stat chain for all 4 chunks at once
                    # (wide ops; one indicator-matmul pair for all chunks)
                    rsall = statp.tile([P, CCH, 2], F32, tag="rsall")
                    nc.vector.tensor_mul(
                        out=rsall[:, :, 1:2], in0=mvall[:, :, 0:1], in1=mvall[:, :, 0:1]
                    )
                    nc.vector.tensor_add(
                        out=rsall[:, :, 1:2], in0=rsall[:, :, 1:2], in1=mvall[:, :, 1:2]
                    )
                    nc.vector.tensor_copy(out=rsall[:, :, 0:1], in_=mvall[:, :, 0:1])
                    gps = app.tile([8, CCH, 2], F32, tag="g", bufs=1)
                    nc.tensor.matmul(gps, lhsT=ind_t, rhs=rsall, start=True, stop=True)
                    gsb = statp.tile([8, CCH, 2], F32, tag="gsb")
                    nc.vector.tensor_copy(out=gsb, in_=gps)
                    rps = app.tile([P, CCH, 2], F32, tag="r", bufs=1)
                    nc.tensor.matmul(rps, lhsT=ind2_t, rhs=gsb, start=True, stop=True)
                    gmall = statp.tile([P, CCH], F32, tag="gmall")
                    gvall = statp.tile([P, CCH], F32, tag="gvall")
                    nc.vector.tensor_copy(out=gmall, in_=rps[:, :, 0:1])
                    nc.vector.tensor_mul(out=gvall, in0=gmall, in1=gmall)
                    nc.vector.tensor_sub(out=gvall, in0=rps[:, :, 1:2], in1=gvall)
                    # rstd = 1/sqrt(var + eps)
                    nc.scalar.activation(out=gvall, in_=gvall, func=AF.Sqrt, bias=eps_t)
                    nc.vector.reciprocal(out=gvall, in_=gvall)
                    nc.vector.tensor_mul(out=A_t, in0=gvall, in1=gnw_t)
                    nc.vector.tensor_mul(out=gmall, in0=gmall, in1=A_t)
                    nc.vector.tensor_sub(out=B_t, in0=gnb_t, in1=gmall)
                    nc.vector.tensor_scalar_mul(out=A64_t, in0=A_t, scalar1=64.0)
                    for wt, w8 in ((wf_q, w8q), (wf_k, w8k), (wf_v, w8v)):
                        for ci in range(CCH):
                            nc.gpsimd.tensor_scalar_mul(
                                out=w8[:, ci, :],
                                in0=wt[:, ts(ci, C)],
                                scalar1=A64_t[:, ci : ci + 1],
                            )

                    # Fold B into projection biases:
                    #   bq2[o] = bq[o] + sum_c wqT[c,o] * B[c]   (same for bk2)
                    #   bv2c[e] = bv[e] + sum_c wvT[c,e] * B[c]
                    for (wt, b_in, b_out) in ((wf_q, bq_t, bq2_t), (wf_k, bk_t, bk2_t)):
                        for oj in range(CCH):
                            bc = app.tile([P, 1], F32, tag="bc", bufs=2)
                            for ci in range(CCH):
                                nc.tensor.matmul(
                                    bc,
                                    lhsT=wt[:, ts(ci * CCH + oj, P)],
                                    rhs=B_t[:, ci : ci + 1],
                                    start=(ci == 0),
                                    stop=(ci == CCH - 1),
                                )
                            nc.vector.tensor_add(
                                out=b_out[:, oj : oj + 1],
                                in0=bc,
                                in1=b_in[:, oj : oj + 1],
                            )
                    for e4 in range(CCH):
                        bc = app.tile([P, 1], F32, tag="bc", bufs=2)
                        for ci in range(CCH):
                            nc.tensor.matmul(
                                bc,
                                lhsT=wf_v[:, ts(ci * CCH + e4, P)],
                                rhs=B_t[:, ci : ci + 1],
                                start=(ci == 0),
                                stop=(ci == CCH - 1),
                            )
                        nc.vector.tensor_add(
                            out=bv2c_t[:, e4 : e4 + 1],
                            in0=bc,
                            in1=bv_t[:, e4 : e4 + 1],
                        )

                # ---------- Phase B: projections (consume raw fp8 x) -------
                DR = mybir.MatmulPerfMode.DoubleRow
                INV64 = 1.0 / 64.0
                with tc.tile_pool(name="bpsum", bufs=4, space="PSUM") as bpp:
                    for nb in range(NB):
                        for oj in range(CCH):
                            kp = bpp.tile([P, 512], F32, tag="pj")
                            for c2 in range(2):
                                nc.tensor.matmul(
                                    kp,
                                    lhsT=w8k[:, 2 * c2 : 2 * c2 + 2, ts(oj, P)],
                                    rhs=xb16[:, 2 * c2 : 2 * c2 + 2, ts(nb, 512)],
                                    start=(c2 == 0),
                                    stop=(c2 == 1),
                                    perf_mode=DR,
                                )
                            if oj % 2 == 0:
                                nc.scalar.activation(
                                    out=k_t[:, oj, ts(nb, 512)],
                                    in_=kp,
                                    func=AF.Identity,
                                    bias=bk2_t[:, oj : oj + 1],
                                    scale=INV64,
                                )
                            else:
                                nc.vector.tensor_scalar(
                                    out=k_t[:, oj, ts(nb, 512)],
                                    in0=kp,
                                    scalar1=INV64,
                                    scalar2=bk2_t[:, oj : oj + 1],
                                    op0=OP.mult,
                                    op1=OP.add,
                                )
                        for mj in range(4):
                            vp = bpp.tile([P, 512], F32, tag="pj")
                            for c2 in range(2):
                                nc.tensor.matmul(
                                    vp,
                                    lhsT=xb16[
                                        :, 2 * c2 : 2 * c2 + 2, ts(nb * 4 + mj, P)
                                    ],
                                    rhs=w8v[:, 2 * c2 : 2 * c2 + 2, :],
                                    start=(c2 == 0),
                                    stop=(c2 == 1),
                                    perf_mode=DR,
                                )
                            if mj % 2 == 0:
                                nc.scalar.activation(
                                    out=vT_t[:, nb * 4 + mj, :],
                                    in_=vp,
                                    func=AF.Identity,
                                    bias=zero_t,
                                    scale=INV64,
                                )
                            else:
                                nc.vector.tensor_scalar_mul(
                                    out=vT_t[:, nb * 4 + mj, :],
                                    in0=vp,
                                    scalar1=INV64,
                                )
                    for nb in range(NBH):
                        for oj in range(CCH):
                            qp = bpp.tile([P, 512], F32, tag="pj")
                            for c2 in range(2):
                                nc.tensor.matmul(
                                    qp,
                                    lhsT=w8q[:, 2 * c2 : 2 * c2 + 2, ts(oj, P)],
                                    rhs=xb16[:, 2 * c2 : 2 * c2 + 2, ts(nb, 512)],
                                    start=(c2 == 0),
                                    stop=(c2 == 1),
                                    perf_mode=DR,
                                )
                            nc.scalar.activation(
                                out=q_t[:, oj, ts(nb, 512)],
                                in_=qp,
                                func=AF.Identity,
                                bias=bq2_t[:, oj : oj + 1],
                                scale=INV64,
                            )

            # ---------- Phase C: attention + out-proj + residual ------------
            _phase_c_fp8(
                nc, tc, k_t, q_t, vT_t, w_o16, xb_t, ones8_t, zero_t, bv2c_t,
                out_p,
            )

    nc.compile()
    return nc


def _phase_c_fp8(nc, tc, k_t, q_t, vT_t, w_o16, xb_t, ones8_t, zero_t, bv2c_t, out_p):
    """Attention with fp8e4 DoubleRow matmuls, PV/Z processed in m-chunk
    pairs. Z is accumulated on the TensorEngine via a DoubleRow ones-matmul
    sharing the PV accumulation loop."""
    DR = mybir.MatmulPerfMode.DoubleRow
    with (
        tc.tile_pool(name="sps", bufs=3, space="PSUM") as sps,
        tc.tile_pool(name="ozp", bufs=4, space="PSUM") as ozp,
        tc.tile_pool(name="zps", bufs=1, space="PSUM") as zpsp,
        tc.tile_pool(name="att", bufs=4) as attp,
        tc.tile_pool(name="fin", bufs=3) as finp,
    ):
        total = NBH * MC
        sp_tiles = {}
        next_s = [0]

        def emit_s(t):
            ns_, mc_ = divmod(t, MC)
            sp = sps.tile([P, 512], F32, tag="s", name=f"s{t}")
            for c2 in range(2):
                nc.tensor.matmul(
                    sp,
                    lhsT=k_t[:, 2 * c2 : 2 * c2 + 2, ts(mc_, P)],
                    rhs=q_t[:, 2 * c2 : 2 * c2 + 2, ts(ns_, 512)],
                    start=(c2 == 0),
                    stop=(c2 == 1),
                    perf_mode=DR,
                )
            sp_tiles[t] = sp

        def ensure_s(upto):
            while next_s[0] < min(upto, total):
                emit_s(next_s[0])
                next_s[0] += 1

        ops = None
        zps = None
        e8 = None
        for t in range(total):
            ns, mc = divmod(t, MC)
            a, j = divmod(mc, 2)
            if mc == 0:
                ops = [
                    ozp.tile([P, 512], F32, tag="oz", name=f"o{ns}_{e4}")
                    for e4 in range(CCH)
                ]
                zps = zpsp.tile([P, 512], F32, tag="z", name=f"z{ns}")
            ensure_s(t + 3)
            if j == 0:
                e8 = attp.tile([P, 2, 512], FP8, tag="e", name=f"e{t}")
            nc.scalar.activation(
                out=e8[:, j, :],
                in_=sp_tiles.pop(t),
                func=AF.Exp,
                bias=zero_t,
                scale=SCALE,
            )
            if j == 1:
                for e4 in range(CCH):
                    nc.tensor.matmul(
                        ops[e4],
                        lhsT=vT_t[:, 2 * a : 2 * a + 2, ts(e4, P)],
                        rhs=e8,
                        start=(a == 0),
                        stop=(a == MC // 2 - 1),
                        perf_mode=DR,
                    )
                nc.tensor.matmul(
                    zps,
                    lhsT=ones8_t,
                    rhs=e8,
                    start=(a == 0),
                    stop=(a == MC // 2 - 1),
                    perf_mode=DR,
                )
            if mc == MC - 1:
                ensure_s(t + 3)
                rz = attp.tile([P, 512], F32, tag="rz", name=f"rz{ns}")
                nc.vector.reciprocal(out=rz, in_=zps)
                osb = attp.tile([P, CCH, 512], BF16, tag="osb", name=f"ob{ns}")
                for e4 in range(CCH):
                    nc.vector.tensor_mul(out=osb[:, e4, :], in0=ops[e4], in1=rz)
                    # v-projection bias, deferred past the PV matmul:
                    # osb = (PV/Z) + bv2c  (exact since sum_m p = 1)
                    nc.vector.tensor_scalar_add(
                        out=osb[:, e4, :],
                        in0=osb[:, e4, :],
                        scalar1=bv2c_t[:, e4 : e4 + 1],
                    )
                for oj in range(CCH):
                    pp = sps.tile([P, 512], F32, tag="s", name=f"pp{ns}_{oj}")
                    for e4 in range(CCH):
                        nc.tensor.matmul(
                            pp,
                            lhsT=w_o16[:, ts(e4 * CCH + oj, P)],
                            rhs=osb[:, e4, :],
                            start=(e4 == 0),
                            stop=(e4 == CCH - 1),
                        )
                    res = finp.tile([P, 512], F32, tag="res", name=f"r{ns}_{oj}")
                    nc.vector.tensor_add(
                        out=res, in0=pp, in1=xb_t[:, oj, ts(ns, 512)]
                    )
                    nc.sync.dma_start(out=out_p[oj, :, ts(ns, 512)], in_=res)


def _pack_blobs(inputs):
    """Build the 8 per-core [P, W] f32 blobs."""
    x = np.asarray(inputs["x"], np.float32)  # [4, 512, 64, 64]
    B = x.shape[0]
    xf = np.ascontiguousarray(x.reshape(B, CCH, P, N))

    def wt(w):
        # w: [o, c] -> lhsT layout [c, o] chunked by c: [P, CCH*C]
        return np.ascontiguousarray(
            np.asarray(w, np.float32).T.reshape(CCH, P, C).transpose(1, 0, 2)
        ).reshape(P, CCH * C)

    def colvec(b):
        return np.ascontiguousarray(
            np.asarray(b, np.float32).reshape(CCH, P).T
        )

    const = np.zeros((P, W - WQOFF), np.float32)
    o = 0
    for nm in ("wq", "wk", "wv", "wo"):
        const[:, o : o + CCH * C] = wt(inputs[nm])
        o += CCH * C
    for nm in ("bq", "bk", "bo", "bv", "gn_w", "gn_b"):
        const[:, o : o + CCH] = colvec(inputs[nm])
        o += CCH
    ind = np.zeros((P, 8), np.float32)
    ind[np.arange(P), np.arange(P) // 16] = 1.0 / 16.0
    const[:, o : o + 8] = ind
    o += 8
    ind2 = np.zeros((8, P), np.float32)
    ind2[np.arange(P) // 16, np.arange(P)] = 1.0
    const[0:8, o : o + P] = ind2

    blobs = []
    for core in range(8):
        b, half = core // 2, core % 2
        blob = np.empty((P, W), np.float32)
        for ci in range(CCH):
            blob[:, ci * N : ci * N + NHALF] = xf[
                b, ci, :, half * NHALF : (half + 1) * NHALF
            ]
            blob[:, ci * N + NHALF : (ci + 1) * N] = xf[
                b, ci, :, (1 - half) * NHALF : (2 - half) * NHALF
            ]
        blob[:, WQOFF:] = const
        blobs.append(blob)
    return blobs


def kernel(**inputs):
    global LAST_EXEC_TIME_NS, _CACHED_NC, _last_in_maps
    x = np.asarray(inputs["x"], np.float32)  # [4, 512, 64, 64]
    B = x.shape[0]
    assert x.shape == (4, C, 64, 64)

    if _CACHED_NC is None:
        _CACHED_NC = build_nc()
    nc = _CACHED_NC

    in_maps = [{"blob": blob} for blob in _pack_blobs(inputs)]

    _last_in_maps = in_maps
    res = run_bass_kernel_spmd(nc, in_maps, core_ids=list(range(8)))
    LAST_EXEC_TIME_NS = res.exec_time_ns

    out = np.empty((B, C, N), np.float32)
    for core in range(8):
        b, half = core // 2, core % 2
        out[b, :, half * NHALF : (half + 1) * NHALF] = (
            res.results[core]["out"].reshape(C, NHALF)
        )
    return out.reshape(B, C, 64, 64)
